# revision 3
# baseline (speedup 1.0000x reference)
"""Bass/Tile TRN2 kernel for nn_AttentionHead: single-head attention with
q/k/v projections (512->64), key mask, softmax over 4096 keys.

Sharding: 8 cores; core c handles batch c//2, query-half c%2 (2048 queries),
with that batch's full k/v replicated. No collectives.

v2: host pre-casts q/k/v to bf16 and pre-transposes to [d, t] layout, so the
device does no cast-DMA and no input transposes. Projections use PE column
packing (tile_position (0,0)/(0,64)) to produce the partition-duplicated
QT/KT needed for row-packed score matmuls in a single moving pass, evacuated
by one DVE op (ACT engine is reserved exclusively for exp).

Per-core dataflow:
  - HWDGE loads qT/kT/vT bf16 [128, 4, t] tiles (d on partitions)
  - QT/KT [128, t] = W^T x^T duplicated on partitions 64-127 (col packing)
  - V via V^T then PE transpose -> V1 [t2, 65]; column 64 holds the key mask
    and V rows are pre-multiplied by the mask (masked softmax ==
    sum(mask*exp*V) / sum(mask*exp), no -1e9 bias needed)
  - scores: S^T tiles [t2=128, 1024] = KT_chunk.T @ QT (contract e=64);
    chunk pairs run concurrently in array row groups 0-63/64-127
  - ScalarE: exp(0.125 * S^T), one call per [128, 1024] psum pair
  - PV: O^T[65, 512] += V1_chunk.T @ expS (row 64 = denominator); PV matmuls
    are emitted one tile behind the scores so the in-order PE never stalls
  - epilogue: PE transpose [65,128] blocks, reciprocal + scale on VectorE
  - query blocks 0,1 stream with the k/v blocks; 2,3 run after from
    SBUF-resident KT/V1 (PSUM bank budget: 4 scores + 2 PV + 2 proj)
"""

import sys
import types

import numpy as np
import ml_dtypes

import concourse.bass as bass
import concourse.tile as tile
from concourse import bacc, mybir
from concourse.masks import make_identity

B, T1, T2, D, E = 4, 4096, 4096, 512, 64
P = 128
T1L = T1 // 2          # queries per core
DC = D // P            # 4 d-chunks
NT2 = T2 // P          # 32 key chunks
TB = 512               # block size (t rows) for proj / k-v streaming
NBLK = T2 // TB        # 8 k/v blocks
NQB = T1L // TB        # 4 query blocks
CPQ = 2                # key chunks per score/exp tile
QW = CPQ * TB          # 1024
F32 = mybir.dt.float32
BF16 = mybir.dt.bfloat16
EXPF = mybir.ActivationFunctionType.Exp


def _install_ntff_hook():
    """Make trace=True usable under axon when antenv.axon_hooks is absent."""
    try:
        import antenv.axon_hooks  # noqa: F401
        return
    except ImportError:
        pass
    try:
        from trn_agent_boot.trn_boot import _ntff_profile_via_ctypes
        hook = _ntff_profile_via_ctypes("/opt/axon/libaxon_pjrt.so")
    except Exception:
        hook = None
    mod = types.ModuleType("antenv.axon_hooks")
    mod.get_axon_ntff_profile_hook = lambda: hook
    mod.set_axon_ntff_profile_hook = lambda h: None
    sys.modules["antenv.axon_hooks"] = mod


def build_body(tc, nc, qT, kT, vT, mask, Wq, bq2, Wk, bk2, Wv, bv, out):
    with (
        tc.tile_pool(name="consts", bufs=1) as consts,
        tc.tile_pool(name="persist", bufs=1) as persist,
    ):
        bq_s = consts.tile([P, 1], F32)
        nc.sync.dma_start(out=bq_s, in_=bq2[:, None])
        # warmup exp so the ACT table set loads during the prologue DMAs
        warm = consts.tile([P, 1], BF16)
        nc.scalar.activation(out=warm, in_=bq_s, func=EXPF, scale=1.0)

        ident_b = consts.tile([P, P], BF16)
        make_identity(nc, ident_b)
        ident_f = consts.tile([P, P], F32)
        make_identity(nc, ident_f)

        wq_b = consts.tile([P, DC, E], BF16)
        nc.sync.dma_start(out=wq_b, in_=Wq.rearrange("(c p) e -> p c e", p=P))
        wk_b = consts.tile([P, DC, E], BF16)
        nc.sync.dma_start(out=wk_b, in_=Wk.rearrange("(c p) e -> p c e", p=P))
        wv_b = consts.tile([P, DC, E], BF16)
        nc.sync.dma_start(out=wv_b, in_=Wv.rearrange("(c p) e -> p c e", p=P))

        bk_s = consts.tile([P, 1], F32)
        nc.sync.dma_start(out=bk_s, in_=bk2[:, None])
        bv_s = consts.tile([E, 1], F32)
        nc.sync.dma_start(out=bv_s, in_=bv[:, None])

        # mask values per key, [partition = t2 % 128, col = t2 // 128]
        mk = consts.tile([P, NT2], F32)
        nc.sync.dma_start(out=mk, in_=mask.rearrange("(c p) -> p c", p=P))

        QT = persist.tile([P, T1L], BF16)
        KT = persist.tile([P, T2], BF16)
        V1 = persist.tile([P, NT2, E + 1], BF16)
        out_sb = persist.tile([P, T1L // P, E], F32)

        # the "ones" column of V1 carries the mask directly: the masked
        # softmax denominator is sum(mask * exp)
        nc.vector.tensor_copy(out=V1[:, :, E], in_=mk)

        qsr = qT.rearrange("(c p) t -> p c t", p=P)
        ksr = kT.rearrange("(c p) t -> p c t", p=P)
        vsr = vT.rearrange("(c p) t -> p c t", p=P)
        orr = out.rearrange("(n p) e -> p n e", p=P)

        pv_tiles = {}
        pending = []

        with (
            tc.tile_pool(name="expp", bufs=4) as expp,
            tc.tile_pool(name="ep", bufs=3) as ep,
            tc.tile_pool(name="psS", bufs=2, space="PSUM") as psS,
            tc.tile_pool(name="psPV", bufs=1, space="PSUM") as psPV,
        ):
            def emit_pv(item):
                qb, qi, ex = item
                for u in range(CPQ):
                    c = CPQ * qi + u
                    nc.tensor.matmul(
                        pv_tiles[qb], V1[:, c, :], ex[:, u * TB:(u + 1) * TB],
                        start=(c == 0), stop=(c == NT2 - 1))

            def scores_exp_pv(qb, qi):
                q0 = qb * TB
                ps = psS.tile([P, QW], F32, tag="s", name=f"s_{qb}_{qi}")
                for u in range(CPQ):
                    c = CPQ * qi + u
                    rg = E * (u % 2)
                    nc.tensor.matmul(
                        ps[:, u * TB:(u + 1) * TB],
                        KT[rg:rg + E, c * P:(c + 1) * P],
                        QT[rg:rg + E, q0:q0 + TB], start=True, stop=True,
                        tile_position=(rg, 0))
                ex = expp.tile([P, QW], BF16, tag="e", name=f"e_{qb}_{qi}")
                nc.scalar.activation(out=ex, in_=ps, func=EXPF, scale=0.125)
                pending.append((qb, qi, ex))
                while len(pending) > 1:
                    emit_pv(pending.pop(0))

            def flush_pv():
                while pending:
                    emit_pv(pending.pop(0))

            def epilogue(qb, psO):
                pvt = pv_tiles.pop(qb)
                q0 = qb * TB
                ov = ep.tile([E + 1, TB], F32, tag="ov", name=f"ov_{qb}")
                nc.vector.tensor_copy(out=ov, in_=pvt)
                for j in range(TB // P):
                    po = psO.tile([P, E + 1], F32, tag="o", name=f"o_{qb}_{j}")
                    nc.tensor.transpose(
                        po, ov[:, j * P:(j + 1) * P],
                        ident_f[0:E + 1, 0:E + 1])
                    rec = ep.tile([P, 1], F32, tag="rec", name=f"rec_{qb}_{j}")
                    nc.vector.reciprocal(rec, po[:, E:E + 1])
                    nc.vector.tensor_scalar_mul(
                        out_sb[:, (q0 + j * P) // P, :], po[:, 0:E], rec)
                n0, n1 = q0 // P, (q0 + TB) // P
                nc.sync.dma_start(out=orr[:, n0:n1, :],
                                  in_=out_sb[:, n0:n1, :])

            # ---------------- stage 1: stream ----------------
            stream_qbs = [0, 1]
            post_qbs = [2, 3]

            with (
                tc.tile_pool(name="qstage", bufs=2) as qstage,
                tc.tile_pool(name="kstage", bufs=NBLK) as kstage,
                tc.tile_pool(name="vstage", bufs=NBLK) as vstage,
                tc.tile_pool(name="psProj", bufs=2, space="PSUM") as psProj,
            ):
                for qb in stream_qbs:
                    pv_tiles[qb] = psPV.tile([E + 1, TB], F32,
                                             tag=f"pv{qb % 2}", name=f"pv_{qb}")

                def proj_dup(st, w_b, b_s, dst, tb):
                    # projection written to partitions 0:64 and duplicated to
                    # 64:128 in one moving pass via PE column packing
                    ps = psProj.tile([P, TB], F32, tag="proj",
                                     name=f"pp_{dst.tensor.name}_{tb}")
                    for j in range(DC):
                        nc.tensor.matmul(
                            ps[0:E, :], w_b[:, j], st[:, j],
                            start=(j == 0), stop=(j == DC - 1),
                            tile_position=(0, 0))
                        nc.tensor.matmul(
                            ps[E:P, :], w_b[:, j], st[:, j],
                            start=(j == 0), stop=(j == DC - 1),
                            tile_position=(0, E))
                    nc.vector.tensor_scalar_add(
                        dst[:, tb * TB:(tb + 1) * TB], ps, b_s)

                # stage-in DMAs: emitted upfront; big bufs means the queues
                # stream at full bandwidth with no rotation stalls
                def stage_in(pool, srcr, nm, tb):
                    st = pool.tile([P, DC, TB], BF16, tag="st",
                                   name=f"st_{nm}_{tb}")
                    nc.sync.dma_start(out=st,
                                      in_=srcr[:, :, tb * TB:(tb + 1) * TB])
                    return st

                qst = stage_in(qstage, qsr, "q", 0)
                kst = {b: stage_in(kstage, ksr, "k", b) for b in range(NBLK)}
                vst = {b: stage_in(vstage, vsr, "v", b) for b in range(NBLK)}

                def vproj_block(tb):
                    st = vst.pop(tb)
                    ps = psProj.tile([E, TB], F32, tag="proj",
                                     name=f"psv_{tb}")
                    for j in range(DC):
                        nc.tensor.matmul(
                            ps, wv_b[:, j], st[:, j],
                            start=(j == 0), stop=(j == DC - 1))
                    vts = ep.tile([E, TB], BF16, tag="vts", name=f"vts_{tb}")
                    nc.vector.tensor_scalar_add(vts, ps, bv_s)
                    pvt = psProj.tile([P, TB // P, E], BF16, tag="proj",
                                      name=f"vt_{tb}")
                    for ci in range(TB // P):
                        nc.tensor.transpose(
                            pvt[:, ci], vts[:, ci * P:(ci + 1) * P],
                            ident_b[0:E, 0:E])
                    for ci in range(TB // P):
                        c = tb * (TB // P) + ci
                        # fold the key mask into V rows: masked softmax
                        # = sum(mask*exp*V) / sum(mask*exp)
                        nc.vector.tensor_scalar_mul(
                            V1[:, c, 0:E], pvt[:, ci], mk[:, c:c + 1])

                # block 0 first so exp starts as early as possible
                proj_dup(qst, wq_b, bq_s, QT, 0)
                proj_dup(kst.pop(0), wk_b, bk_s, KT, 0)
                vproj_block(0)
                scores_exp_pv(0, 0)
                scores_exp_pv(0, 1)
                # remaining query-block projections
                for tb in range(1, NQB):
                    qst = stage_in(qstage, qsr, "q", tb)
                    proj_dup(qst, wq_b, bq_s, QT, tb)
                    if tb == 1:
                        scores_exp_pv(1, 0)
                        scores_exp_pv(1, 1)
                for b in range(1, NBLK):
                    proj_dup(kst.pop(b), wk_b, bk_s, KT, b)
                    vproj_block(b)
                    for qb in stream_qbs:
                        for qi in range(b * CPQ, (b + 1) * CPQ):
                            scores_exp_pv(qb, qi)

            # ---------------- stage 2: remaining query blocks ----------------
            with tc.tile_pool(name="psO", bufs=2, space="PSUM") as psO:
                flush_pv()
                pending_stream_epi = list(stream_qbs)
                # free pv0 so the first post query block can start accumulating
                epilogue(pending_stream_epi.pop(0), psO)

                def drain_stream_epi():
                    while pending_stream_epi:
                        epilogue(pending_stream_epi.pop(0), psO)

                for qb in post_qbs:
                    pv_tiles[qb] = psPV.tile([E + 1, TB], F32,
                                             tag=f"pv{qb % 2}", name=f"pv_{qb}")
                    for qi in range(NT2 // CPQ):
                        scores_exp_pv(qb, qi)
                        if qi >= 1:
                            drain_stream_epi()
                    drain_stream_epi()
                    flush_pv()
                    epilogue(qb, psO)


def build_nc(t1l=T1L, t2=T2):
    nc = bacc.Bacc()
    qT = nc.declare_dram_parameter("qT", [D, t1l], BF16, isOutput=False)
    kT = nc.declare_dram_parameter("kT", [D, t2], BF16, isOutput=False)
    vT = nc.declare_dram_parameter("vT", [D, t2], BF16, isOutput=False)
    mask = nc.declare_dram_parameter("mask", [t2], F32, isOutput=False)
    Wq = nc.declare_dram_parameter("Wq", [D, E], BF16, isOutput=False)
    bq2 = nc.declare_dram_parameter("bq2", [P], F32, isOutput=False)
    Wk = nc.declare_dram_parameter("Wk", [D, E], BF16, isOutput=False)
    bk2 = nc.declare_dram_parameter("bk2", [P], F32, isOutput=False)
    Wv = nc.declare_dram_parameter("Wv", [D, E], BF16, isOutput=False)
    bv = nc.declare_dram_parameter("bv", [E], F32, isOutput=False)
    out = nc.declare_dram_parameter("out", [t1l, E], F32, isOutput=True)
    with tile.TileContext(nc) as tc:
        build_body(tc, nc, qT[:], kT[:], vT[:], mask[:], Wq[:], bq2[:],
                   Wk[:], bk2[:], Wv[:], bv[:], out[:])
    nc.compile()
    return nc


_NC_CACHE = {}


def _get_nc():
    if "nc" not in _NC_CACHE:
        _NC_CACHE["nc"] = build_nc()
    return _NC_CACHE["nc"]


def make_in_maps(q, k, v, mask, Wq, bq, Wk, bk, Wv, bv):
    bf = ml_dtypes.bfloat16
    f32 = np.float32

    def bfT(x):  # cast [t, d] fp32 -> bf16, transpose to [d, t] contiguous
        return np.ascontiguousarray(np.asarray(x, f32).astype(bf).T)

    shared = {
        "Wq": np.ascontiguousarray(np.asarray(Wq, f32).astype(bf)),
        "Wk": np.ascontiguousarray(np.asarray(Wk, f32).astype(bf)),
        "Wv": np.ascontiguousarray(np.asarray(Wv, f32).astype(bf)),
        "bq2": np.concatenate([bq, bq]).astype(f32),
        "bk2": np.concatenate([bk, bk]).astype(f32),
        "bv": np.ascontiguousarray(bv, f32),
    }
    per_b = []
    for b in range(B):
        per_b.append({
            "kT": bfT(k[b]),
            "vT": bfT(v[b]),
            "mask": np.ascontiguousarray(mask[b, 0], f32),
        })
    in_maps = []
    for c in range(8):
        b, h = divmod(c, 2)
        in_maps.append({
            "qT": bfT(q[b, h * T1L:(h + 1) * T1L]),
            **per_b[b],
            **shared,
        })
    return in_maps


def assemble_out(results):
    out = np.empty((B, T1, E), np.float32)
    for c in range(8):
        b, h = divmod(c, 2)
        out[b, h * T1L:(h + 1) * T1L] = results[c]["out"]
    return out


def run(inputs, trace=False):
    from concourse.bass_utils import run_bass_kernel_spmd
    _install_ntff_hook()
    nc = _get_nc()
    in_maps = make_in_maps(**inputs)
    res = run_bass_kernel_spmd(nc, in_maps, list(range(8)), trace=trace)
    return assemble_out(res.results), res


def kernel(q, k, v, mask, Wq, bq, Wk, bk, Wv, bv):
    out, _ = run(dict(q=q, k=k, v=v, mask=mask, Wq=Wq, bq=bq, Wk=Wk, bk=bk,
                      Wv=Wv, bv=bv))
    return out


# revision 4
# speedup vs baseline: 1.0932x; 1.0932x over previous
"""Bass/Tile TRN2 kernel for nn_AttentionHead: single-head attention with
q/k/v projections (512->64), key mask, softmax over 4096 keys.

Sharding: 8 cores; core c handles batch c//2, query-half c%2 (2048 queries),
with that batch's full k/v replicated. No collectives.

v3: host pre-casts q/k/v to bf16, pre-transposes to [d, t] and lays tiles
out p-major ([128, block, chunk, t]) so every DMA is 128 large contiguous
descriptors (cheap HWDGE issue). DMA issue is split across the Sync (q/out/
consts) and GpSimd (k/v) queues so descriptor generation isn't serialized.

Per-core dataflow:
  - HWDGE loads qT/kT/vT bf16 [128, 4, 512] tiles (d on partitions)
  - QT/KT [128, t] = W^T x^T duplicated on partitions 64-127 via PE column
    packing (tile_position (0,0)/(0,64)) in a single moving pass, evacuated
    by one DVE op (ACT engine is reserved exclusively for exp)
  - V via V^T then PE transpose -> V1 [t2, 65]; column 64 holds the key mask
    and V rows are pre-multiplied by the mask (masked softmax ==
    sum(mask*exp*V) / sum(mask*exp), no -1e9 bias needed)
  - scores: S^T tiles [t2=128, 1024] = KT_chunk.T @ QT (contract e=64);
    chunk pairs run concurrently in array row groups 0-63/64-127
  - ScalarE: exp(0.125 * S^T), one call per [128, 1024] psum pair
  - PV: O^T[65, 512] += V1_chunk.T @ expS (row 64 = denominator); PV matmuls
    are emitted one tile behind the scores so the in-order PE never stalls
  - epilogue: PE transpose [65,128] blocks, reciprocal + scale on VectorE
  - query blocks 0,1 stream with the k/v blocks; 2,3 run after from
    SBUF-resident KT/V1 (PSUM bank budget: 4 scores + 2 PV + 2 proj)
"""

import sys
import types

import numpy as np
import ml_dtypes

import concourse.bass as bass
import concourse.tile as tile
from concourse import bacc, mybir
from concourse.masks import make_identity

B, T1, T2, D, E = 4, 4096, 4096, 512, 64
P = 128
T1L = T1 // 2          # queries per core
DC = D // P            # 4 d-chunks
NT2 = T2 // P          # 32 key chunks
TB = 512               # block size (t rows) for proj / k-v streaming
NBLK = T2 // TB        # 8 k/v blocks
NQB = T1L // TB        # 4 query blocks
CPQ = 2                # key chunks per score/exp tile
QW = CPQ * TB          # 1024
F32 = mybir.dt.float32
BF16 = mybir.dt.bfloat16
EXPF = mybir.ActivationFunctionType.Exp


def _install_ntff_hook():
    """Make trace=True usable under axon when antenv.axon_hooks is absent."""
    try:
        import antenv.axon_hooks  # noqa: F401
        return
    except ImportError:
        pass
    try:
        from trn_agent_boot.trn_boot import _ntff_profile_via_ctypes
        hook = _ntff_profile_via_ctypes("/opt/axon/libaxon_pjrt.so")
    except Exception:
        hook = None
    mod = types.ModuleType("antenv.axon_hooks")
    mod.get_axon_ntff_profile_hook = lambda: hook
    mod.set_axon_ntff_profile_hook = lambda h: None
    sys.modules["antenv.axon_hooks"] = mod


def build_body(tc, nc, qh, kh, vh, mh, Wq, bq2, Wk, bk2, Wv, bv, out):
    with (
        tc.tile_pool(name="consts", bufs=1) as consts,
        tc.tile_pool(name="persist", bufs=1) as persist,
    ):
        bq_s = consts.tile([P, 1], F32)
        nc.sync.dma_start(out=bq_s, in_=bq2[:, None])
        # warmup exp so the ACT table set loads during the prologue DMAs
        warm = consts.tile([P, 1], BF16)
        nc.scalar.activation(out=warm, in_=bq_s, func=EXPF, scale=1.0)

        ident_b = consts.tile([P, P], BF16)
        make_identity(nc, ident_b)
        ident_f = consts.tile([P, P], F32)
        make_identity(nc, ident_f)

        wq_b = consts.tile([P, DC, E], BF16)
        nc.sync.dma_start(out=wq_b, in_=Wq)
        wk_b = consts.tile([P, DC, E], BF16)
        nc.sync.dma_start(out=wk_b, in_=Wk)
        wv_b = consts.tile([P, DC, E], BF16)
        nc.sync.dma_start(out=wv_b, in_=Wv)

        bk_s = consts.tile([P, 1], F32)
        nc.sync.dma_start(out=bk_s, in_=bk2[:, None])
        bv_s = consts.tile([E, 1], F32)
        nc.sync.dma_start(out=bv_s, in_=bv[:, None])

        # mask values per key, [partition = t2 % 128, col = t2 // 128]
        mk = consts.tile([P, NT2], F32)
        nc.sync.dma_start(out=mk, in_=mh)

        QT = persist.tile([P, T1L], BF16)
        KT = persist.tile([P, T2], BF16)
        V1 = persist.tile([P, NT2, E + 1], BF16)
        out_sb = persist.tile([P, T1L // P, E], F32)

        # the "ones" column of V1 carries the mask directly: the masked
        # softmax denominator is sum(mask * exp)
        nc.vector.tensor_copy(out=V1[:, :, E], in_=mk)

        pv_tiles = {}
        pending = []

        with (
            tc.tile_pool(name="expp", bufs=4) as expp,
            tc.tile_pool(name="ep", bufs=3) as ep,
            tc.tile_pool(name="psS", bufs=2, space="PSUM") as psS,
            tc.tile_pool(name="psPV", bufs=1, space="PSUM") as psPV,
        ):
            def emit_pv(item):
                qb, qi, ex = item
                for u in range(CPQ):
                    c = CPQ * qi + u
                    nc.tensor.matmul(
                        pv_tiles[qb], V1[:, c, :], ex[:, u * TB:(u + 1) * TB],
                        start=(c == 0), stop=(c == NT2 - 1))

            def scores_exp_pv(qb, qi):
                q0 = qb * TB
                ps = psS.tile([P, QW], F32, tag="s", name=f"s_{qb}_{qi}")
                for u in range(CPQ):
                    c = CPQ * qi + u
                    rg = E * (u % 2)
                    nc.tensor.matmul(
                        ps[:, u * TB:(u + 1) * TB],
                        KT[rg:rg + E, c * P:(c + 1) * P],
                        QT[rg:rg + E, q0:q0 + TB], start=True, stop=True,
                        tile_position=(rg, 0))
                ex = expp.tile([P, QW], BF16, tag="e", name=f"e_{qb}_{qi}")
                nc.scalar.activation(out=ex, in_=ps, func=EXPF, scale=0.125)
                pending.append((qb, qi, ex))
                while len(pending) > 1:
                    emit_pv(pending.pop(0))

            def flush_pv():
                while pending:
                    emit_pv(pending.pop(0))

            def epilogue(qb, psO):
                pvt = pv_tiles.pop(qb)
                q0 = qb * TB
                ov = ep.tile([E + 1, TB], F32, tag="ov", name=f"ov_{qb}")
                nc.vector.tensor_copy(out=ov, in_=pvt)
                for j in range(TB // P):
                    po = psO.tile([P, E + 1], F32, tag="o", name=f"o_{qb}_{j}")
                    nc.tensor.transpose(
                        po, ov[:, j * P:(j + 1) * P],
                        ident_f[0:E + 1, 0:E + 1])
                    rec = ep.tile([P, 1], F32, tag="rec", name=f"rec_{qb}_{j}")
                    nc.vector.reciprocal(rec, po[:, E:E + 1])
                    nc.vector.tensor_scalar_mul(
                        out_sb[:, (q0 + j * P) // P, :], po[:, 0:E], rec)
                n0, n1 = q0 // P, (q0 + TB) // P
                nc.sync.dma_start(out=out[:, n0:n1, :],
                                  in_=out_sb[:, n0:n1, :])

            # ---------------- stage 1: stream ----------------
            stream_qbs = [0, 1]
            post_qbs = [2, 3]

            with (
                tc.tile_pool(name="qstage", bufs=NQB) as qstage,
                tc.tile_pool(name="kstage", bufs=NBLK) as kstage,
                tc.tile_pool(name="vstage", bufs=NBLK) as vstage,
                tc.tile_pool(name="psProj", bufs=2, space="PSUM") as psProj,
            ):
                for qb in stream_qbs:
                    pv_tiles[qb] = psPV.tile([E + 1, TB], F32,
                                             tag=f"pv{qb % 2}", name=f"pv_{qb}")

                # stage-in DMAs all emitted upfront: q on the sync queue,
                # k/v interleaved per block on the gpsimd queue
                def stage_in(pool, src, nm, tb, eng):
                    st = pool.tile([P, DC, TB], BF16, tag="st",
                                   name=f"st_{nm}_{tb}")
                    eng.dma_start(out=st, in_=src[:, tb])
                    return st

                qst = {tb: stage_in(qstage, qh, "q", tb, nc.sync)
                       for tb in range(NQB)}
                kst, vst = {}, {}
                for b in range(NBLK):
                    kst[b] = stage_in(kstage, kh, "k", b, nc.gpsimd)
                    vst[b] = stage_in(vstage, vh, "v", b, nc.gpsimd)

                def proj_dup(st, w_b, b_s, dst, tb):
                    # projection written to partitions 0:64 and duplicated to
                    # 64:128 in one moving pass via PE column packing
                    ps = psProj.tile([P, TB], F32, tag="proj",
                                     name=f"pp_{dst.tensor.name}_{tb}")
                    for j in range(DC):
                        nc.tensor.matmul(
                            ps[0:E, :], w_b[:, j], st[:, j],
                            start=(j == 0), stop=(j == DC - 1),
                            tile_position=(0, 0))
                        nc.tensor.matmul(
                            ps[E:P, :], w_b[:, j], st[:, j],
                            start=(j == 0), stop=(j == DC - 1),
                            tile_position=(0, E))
                    nc.vector.tensor_scalar_add(
                        dst[:, tb * TB:(tb + 1) * TB], ps, b_s)

                def vproj_block(tb):
                    st = vst.pop(tb)
                    ps = psProj.tile([E, TB], F32, tag="proj",
                                     name=f"psv_{tb}")
                    for j in range(DC):
                        nc.tensor.matmul(
                            ps, wv_b[:, j], st[:, j],
                            start=(j == 0), stop=(j == DC - 1))
                    vts = ep.tile([E, TB], BF16, tag="vts", name=f"vts_{tb}")
                    nc.vector.tensor_scalar_add(vts, ps, bv_s)
                    pvt = psProj.tile([P, TB // P, E], BF16, tag="proj",
                                      name=f"vt_{tb}")
                    for ci in range(TB // P):
                        nc.tensor.transpose(
                            pvt[:, ci], vts[:, ci * P:(ci + 1) * P],
                            ident_b[0:E, 0:E])
                    for ci in range(TB // P):
                        c = tb * (TB // P) + ci
                        # fold the key mask into V rows: masked softmax
                        # = sum(mask*exp*V) / sum(mask*exp)
                        nc.vector.tensor_scalar_mul(
                            V1[:, c, 0:E], pvt[:, ci], mk[:, c:c + 1])

                # block 0 first so exp starts as early as possible
                proj_dup(qst.pop(0), wq_b, bq_s, QT, 0)
                proj_dup(kst.pop(0), wk_b, bk_s, KT, 0)
                vproj_block(0)
                scores_exp_pv(0, 0)
                scores_exp_pv(0, 1)
                # remaining query-block projections
                for tb in range(1, NQB):
                    proj_dup(qst.pop(tb), wq_b, bq_s, QT, tb)
                    if tb == 1:
                        scores_exp_pv(1, 0)
                        scores_exp_pv(1, 1)
                for b in range(1, NBLK):
                    proj_dup(kst.pop(b), wk_b, bk_s, KT, b)
                    vproj_block(b)
                    for qb in stream_qbs:
                        for qi in range(b * CPQ, (b + 1) * CPQ):
                            scores_exp_pv(qb, qi)

            # ---------------- stage 2: remaining query blocks ----------------
            with tc.tile_pool(name="psO", bufs=2, space="PSUM") as psO:
                flush_pv()
                pending_stream_epi = list(stream_qbs)
                # free pv0 so the first post query block can start accumulating
                epilogue(pending_stream_epi.pop(0), psO)

                def drain_stream_epi():
                    while pending_stream_epi:
                        epilogue(pending_stream_epi.pop(0), psO)

                for qb in post_qbs:
                    pv_tiles[qb] = psPV.tile([E + 1, TB], F32,
                                             tag=f"pv{qb % 2}", name=f"pv_{qb}")
                    for qi in range(NT2 // CPQ):
                        scores_exp_pv(qb, qi)
                        if qi >= 1:
                            drain_stream_epi()
                    drain_stream_epi()
                    flush_pv()
                    epilogue(qb, psO)


def build_nc(t1l=T1L, t2=T2):
    nc = bacc.Bacc()
    qh = nc.declare_dram_parameter("qh", [P, NQB, DC, TB], BF16, isOutput=False)
    kh = nc.declare_dram_parameter("kh", [P, NBLK, DC, TB], BF16,
                                   isOutput=False)
    vh = nc.declare_dram_parameter("vh", [P, NBLK, DC, TB], BF16,
                                   isOutput=False)
    mh = nc.declare_dram_parameter("mh", [P, NT2], F32, isOutput=False)
    Wq = nc.declare_dram_parameter("Wq", [P, DC, E], BF16, isOutput=False)
    bq2 = nc.declare_dram_parameter("bq2", [P], F32, isOutput=False)
    Wk = nc.declare_dram_parameter("Wk", [P, DC, E], BF16, isOutput=False)
    bk2 = nc.declare_dram_parameter("bk2", [P], F32, isOutput=False)
    Wv = nc.declare_dram_parameter("Wv", [P, DC, E], BF16, isOutput=False)
    bv = nc.declare_dram_parameter("bv", [E], F32, isOutput=False)
    out = nc.declare_dram_parameter("out", [P, T1L // P, E], F32,
                                    isOutput=True)
    with tile.TileContext(nc) as tc:
        build_body(tc, nc, qh[:], kh[:], vh[:], mh[:], Wq[:], bq2[:],
                   Wk[:], bk2[:], Wv[:], bv[:], out[:])
    nc.compile()
    return nc


_NC_CACHE = {}


def _get_nc():
    if "nc" not in _NC_CACHE:
        _NC_CACHE["nc"] = build_nc()
    return _NC_CACHE["nc"]


def make_in_maps(q, k, v, mask, Wq, bq, Wk, bk, Wv, bv):
    bf = ml_dtypes.bfloat16
    f32 = np.float32

    def xh(x, nb):  # [t, d] fp32 -> [128, nb, 4, 512] bf16 p-major
        xt = np.asarray(x, f32).astype(bf).T            # [512, t]
        xt = xt.reshape(DC, P, nb, TB).transpose(1, 2, 0, 3)
        return np.ascontiguousarray(xt)

    def wh(W):  # [512, 64] fp32 -> [128, 4, 64] bf16 p-major
        Wr = np.asarray(W, f32).astype(bf).reshape(DC, P, E).transpose(1, 0, 2)
        return np.ascontiguousarray(Wr)

    shared = {
        "Wq": wh(Wq), "Wk": wh(Wk), "Wv": wh(Wv),
        "bq2": np.concatenate([bq, bq]).astype(f32),
        "bk2": np.concatenate([bk, bk]).astype(f32),
        "bv": np.ascontiguousarray(bv, f32),
    }
    per_b = []
    for b in range(B):
        per_b.append({
            "kh": xh(k[b], NBLK),
            "vh": xh(v[b], NBLK),
            "mh": np.ascontiguousarray(
                np.asarray(mask[b, 0], f32).reshape(NT2, P).T),
        })
    in_maps = []
    for c in range(8):
        b, h = divmod(c, 2)
        in_maps.append({
            "qh": xh(q[b, h * T1L:(h + 1) * T1L], NQB),
            **per_b[b],
            **shared,
        })
    return in_maps


def assemble_out(results):
    out = np.empty((B, T1, E), np.float32)
    for c in range(8):
        b, h = divmod(c, 2)
        # device out is [128, 16, 64] p-major -> [2048, 64]
        o = results[c]["out"].transpose(1, 0, 2).reshape(T1L, E)
        out[b, h * T1L:(h + 1) * T1L] = o
    return out


def run(inputs, trace=False):
    from concourse.bass_utils import run_bass_kernel_spmd
    _install_ntff_hook()
    nc = _get_nc()
    in_maps = make_in_maps(**inputs)
    res = run_bass_kernel_spmd(nc, in_maps, list(range(8)), trace=trace)
    return assemble_out(res.results), res


def kernel(q, k, v, mask, Wq, bq, Wk, bk, Wv, bv):
    out, _ = run(dict(q=q, k=k, v=v, mask=mask, Wq=Wq, bq=bq, Wk=Wk, bk=bk,
                      Wv=Wv, bv=bv))
    return out


# revision 10
# speedup vs baseline: 1.2317x; 1.1267x over previous
"""Bass/Tile TRN2 kernel for nn_AttentionHead: single-head attention with
q/k/v projections (512->64), key mask, softmax over 4096 keys.

Sharding: 8 cores; core c handles batch c//2, query-half c%2 (2048 queries),
with that batch's full k/v replicated. No collectives.

v4: host pre-casts to bf16, pre-transposes to [d, t], lays tiles out p-major
([128, block, chunk, t]) so every DMA is 128 contiguous 4KB descriptors, and
issues DMAs only on the two HWDGE rings (sync + scalar; gpsimd SWDGE was
serializing v3's prologue). All PE transposes are replaced by DMA XBAR
transposes: v and q rows are host-permuted within each 512-block by
PERM[j] = (j%4)*128 + j//4, which makes the [80,512] -> [128,4,80] XBAR
transpose land V's natural layout (and the q permutation cancels exactly
through the output transpose, so assembly is unchanged).

Per-core dataflow:
  - HWDGE loads qT/kT/vT bf16 [128, 4, 512] tiles (d on partitions)
  - QT/KT [128, t] = W^T x^T duplicated on partitions 64-127 via PE column
    packing (tile_position (0,0)/(0,64)) in one moving pass, evacuated by
    one DVE op (ACT engine is reserved exclusively for exp)
  - V: V^T proj psum -> one DVE scalar_tensor_tensor (+bias, *mask) ->
    [80, 512] staging (row 64 = mask row) -> XBAR transpose -> V1
    [t2, 80] whose col 64 is the masked-softmax denominator column
  - scores: S^T tiles [t2=128, 1024] = KT_chunk.T @ QT (contract e=64);
    chunk pairs run concurrently in array row groups 0-63/64-127
  - ScalarE: exp(0.125 * S^T), one call per [128, 1024] psum pair
  - PV: O^T[65, 512] += V1_chunk.T @ expS (row 64 = denominator); PV matmuls
    are emitted one tile behind the scores so the in-order PE never stalls
  - epilogue: psum -> bf16 [80, 512] -> XBAR transpose -> [128, 4, 80],
    reciprocal + scale on VectorE, p-major output DMA
  - query blocks 0,1 stream with the k/v blocks; 2,3 run after from
    SBUF-resident KT/V1 (PSUM bank budget: 4 scores + 2 PV + 2 proj)
"""

import sys
import types

import numpy as np
import ml_dtypes

import concourse.bass as bass
import concourse.tile as tile
from concourse import bacc, mybir
from concourse.masks import make_identity

B, T1, T2, D, E = 4, 4096, 4096, 512, 64
P = 128
T1L = T1 // 2          # queries per core
DC = D // P            # 4 d-chunks
NT2 = T2 // P          # 32 key chunks
TB = 512               # block size (t rows) for proj / k-v streaming
NBLK = T2 // TB        # 8 k/v blocks
NQB = T1L // TB        # 4 query blocks
CPQ = 2                # key chunks per score/exp tile
QW = CPQ * TB          # 1024
VR = 80                # padded row count for XBAR transposes (65 -> 80)
F32 = mybir.dt.float32
BF16 = mybir.dt.bfloat16
EXPF = mybir.ActivationFunctionType.Exp
ADD = mybir.AluOpType.add
MULT = mybir.AluOpType.mult

# within-512-block interleave so the XBAR transpose yields natural layout
PERM = (np.arange(TB) % 4) * P + np.arange(TB) // 4


def _install_ntff_hook():
    """Make trace=True usable under axon when antenv.axon_hooks is absent."""
    try:
        import antenv.axon_hooks  # noqa: F401
        return
    except ImportError:
        pass
    try:
        from trn_agent_boot.trn_boot import _ntff_profile_via_ctypes
        hook = _ntff_profile_via_ctypes("/opt/axon/libaxon_pjrt.so")
    except Exception:
        hook = None
    mod = types.ModuleType("antenv.axon_hooks")
    mod.get_axon_ntff_profile_hook = lambda: hook
    mod.set_axon_ntff_profile_hook = lambda h: None
    sys.modules["antenv.axon_hooks"] = mod


def build_body(tc, nc, qh, kh, vh, mrow, Wq, bq2, Wk, bk2, Wv, bv, out):
    with (
        tc.tile_pool(name="consts", bufs=1) as consts,
        tc.tile_pool(name="persist", bufs=1) as persist,
    ):
        bq_s = consts.tile([P, 1], F32)
        nc.scalar.dma_start(out=bq_s, in_=bq2[:, None])
        # warmup exp so the ACT table set loads during the prologue DMAs
        warm = consts.tile([P, 1], BF16)
        nc.scalar.activation(out=warm, in_=bq_s, func=EXPF, scale=1.0)

        wq_b = consts.tile([P, DC, E], BF16)
        nc.scalar.dma_start(out=wq_b, in_=Wq)
        wk_b = consts.tile([P, DC, E], BF16)
        nc.scalar.dma_start(out=wk_b, in_=Wk)
        wv_b = consts.tile([P, DC, E], BF16)
        nc.scalar.dma_start(out=wv_b, in_=Wv)

        bk_s = consts.tile([P, 1], F32)
        nc.scalar.dma_start(out=bk_s, in_=bk2[:, None])
        bv_s = consts.tile([E, 1], F32)
        nc.scalar.dma_start(out=bv_s, in_=bv[:, None])

        # per-key mask values in staged (block-permuted) order, bf16,
        # replicated across the 64 e-partitions (stride-0 partition DMA)
        mrow_s = consts.tile([E + 1, T2], BF16)
        nc.scalar.dma_start(
            out=mrow_s,
            in_=bass.AP(tensor=mrow.tensor, offset=mrow.offset,
                        ap=[[0, E + 1], mrow.ap[0]]))

        QT = persist.tile([P, T1L], BF16)
        KT = persist.tile([P, T2], BF16)
        V1 = persist.tile([P, NT2, VR], BF16)
        out_sb = persist.tile([P, T1L // P, E], F32)

        pv_tiles = {}
        pending = []

        with (
            tc.tile_pool(name="expp", bufs=4) as expp,
            tc.tile_pool(name="ep", bufs=3) as ep,
            tc.tile_pool(name="psS", bufs=2, space="PSUM") as psS,
            tc.tile_pool(name="psPV", bufs=1, space="PSUM") as psPV,
        ):
            def emit_pv(item):
                qb, qi, ex = item
                for u in range(CPQ):
                    c = CPQ * qi + u
                    nc.tensor.matmul(
                        pv_tiles[qb], V1[:, c, 0:E + 1],
                        ex[:, u * TB:(u + 1) * TB],
                        start=(c == 0), stop=(c == NT2 - 1))

            def scores_exp_pv(qb, qi):
                q0 = qb * TB
                ps = psS.tile([P, QW], F32, tag="s", name=f"s_{qb}_{qi}")
                for u in range(CPQ):
                    c = CPQ * qi + u
                    rg = E * (u % 2)
                    nc.tensor.matmul(
                        ps[:, u * TB:(u + 1) * TB],
                        KT[rg:rg + E, c * P:(c + 1) * P],
                        QT[rg:rg + E, q0:q0 + TB], start=True, stop=True,
                        tile_position=(rg, 0))
                ex = expp.tile([P, QW], BF16, tag="e", name=f"e_{qb}_{qi}")
                nc.scalar.activation(out=ex, in_=ps, func=EXPF, scale=0.125)
                pending.append((qb, qi, ex))
                while len(pending) > 1:
                    emit_pv(pending.pop(0))

            def flush_pv():
                while pending:
                    emit_pv(pending.pop(0))

            def epilogue(qb):
                pvt = pv_tiles.pop(qb)
                q0 = qb * TB
                ov = ep.tile([VR, TB], BF16, tag="ov", name=f"ov_{qb}")
                nc.gpsimd.memset(ov[E:VR, :], 0.0)
                nc.vector.tensor_copy(out=ov[0:E + 1, :], in_=pvt)
                po = ep.tile([P, TB // P, VR], BF16, tag="po",
                             name=f"po_{qb}")
                nc.sync.dma_start_transpose(out=po, in_=ov)
                for j in range(TB // P):
                    rec = ep.tile([P, 1], F32, tag="rec", name=f"rec_{qb}_{j}")
                    nc.vector.reciprocal(rec, po[:, j, E:E + 1])
                    nc.vector.tensor_scalar_mul(
                        out_sb[:, (q0 + j * P) // P, :], po[:, j, 0:E], rec)
                n0, n1 = q0 // P, (q0 + TB) // P
                nc.sync.dma_start(out=out[:, n0:n1, :],
                                  in_=out_sb[:, n0:n1, :])

            # ---------------- stage 1: stream ----------------
            stream_qbs = [0, 1]
            post_qbs = [2, 3]

            with (
                tc.tile_pool(name="qstage", bufs=NQB) as qstage,
                tc.tile_pool(name="kstage", bufs=NBLK) as kstage,
                tc.tile_pool(name="vstage", bufs=NBLK) as vstage,
                tc.tile_pool(name="psProj", bufs=2, space="PSUM") as psProj,
            ):
                for qb in stream_qbs:
                    pv_tiles[qb] = psPV.tile([E + 1, TB], F32,
                                             tag=f"pv{qb % 2}", name=f"pv_{qb}")

                # q on the scalar HWDGE ring (issued before any exp exists);
                # k/v batched on the sync ring, interleaved per block
                qst = {}
                for tb in range(NQB):
                    qst[tb] = qstage.tile([P, DC, TB], BF16, tag="st",
                                          name=f"st_q_{tb}")
                    nc.scalar.dma_start(out=qst[tb], in_=qh[:, tb])

                kst, vst = {}, {}

                def kv_dma(dst, pool, src, nm, b0, nb):
                    tl = pool.tile([P, nb, DC, TB], BF16, tag=f"st_{b0}",
                                   name=f"st_{nm}_{b0}", bufs=1)
                    nc.sync.dma_start(out=tl, in_=src[:, b0:b0 + nb])
                    for i in range(nb):
                        dst[b0 + i] = tl[:, i]

                kv_dma(kst, kstage, kh, "k", 0, 1)
                kv_dma(vst, vstage, vh, "v", 0, 1)
                for b0 in (1, 3, 5):
                    kv_dma(kst, kstage, kh, "k", b0, 2)
                    kv_dma(vst, vstage, vh, "v", b0, 2)
                kv_dma(kst, kstage, kh, "k", 7, 1)
                kv_dma(vst, vstage, vh, "v", 7, 1)

                def proj_dup(st, w_b, b_s, dst, tb):
                    # projection written to partitions 0:64 and duplicated to
                    # 64:128 in one moving pass via PE column packing
                    ps = psProj.tile([P, TB], F32, tag="proj",
                                     name=f"pp_{dst.tensor.name}_{tb}")
                    for j in range(DC):
                        nc.tensor.matmul(
                            ps[0:E, :], w_b[:, j], st[:, j],
                            start=(j == 0), stop=(j == DC - 1),
                            tile_position=(0, 0))
                        nc.tensor.matmul(
                            ps[E:P, :], w_b[:, j], st[:, j],
                            start=(j == 0), stop=(j == DC - 1),
                            tile_position=(0, E))
                    nc.vector.tensor_scalar_add(
                        dst[:, tb * TB:(tb + 1) * TB], ps, b_s)

                def vproj_block(tb):
                    st = vst.pop(tb)
                    ps = psProj.tile([E, TB], F32, tag="proj",
                                     name=f"psv_{tb}")
                    for j in range(DC):
                        nc.tensor.matmul(
                            ps, wv_b[:, j], st[:, j],
                            start=(j == 0), stop=(j == DC - 1))
                    bsl = slice(tb * TB, (tb + 1) * TB)
                    vm = ep.tile([VR, TB], BF16, tag="vm", name=f"vm_{tb}")
                    nc.gpsimd.memset(vm[E:VR, :], 0.0)
                    # fold bias and key mask into V rows in one DVE op:
                    # masked softmax = sum(mask*exp*V) / sum(mask*exp)
                    nc.vector.scalar_tensor_tensor(
                        out=vm[0:E, :], in0=ps, scalar=bv_s,
                        in1=mrow_s[0:E, bsl], op0=ADD, op1=MULT)
                    # row 64 carries the mask itself: the denominator column
                    nc.vector.tensor_copy(out=vm[E:E + 1, :],
                                          in_=mrow_s[E:E + 1, bsl])
                    nc.sync.dma_start_transpose(
                        out=V1[:, tb * (TB // P):(tb + 1) * (TB // P), :],
                        in_=vm)

                # block 0 first so exp starts as early as possible
                proj_dup(qst.pop(0), wq_b, bq_s, QT, 0)
                proj_dup(kst.pop(0), wk_b, bk_s, KT, 0)
                vproj_block(0)
                scores_exp_pv(0, 0)
                scores_exp_pv(0, 1)
                # remaining query-block projections
                for tb in range(1, NQB):
                    proj_dup(qst.pop(tb), wq_b, bq_s, QT, tb)
                    if tb == 1:
                        scores_exp_pv(1, 0)
                        scores_exp_pv(1, 1)
                for b in range(1, NBLK):
                    proj_dup(kst.pop(b), wk_b, bk_s, KT, b)
                    vproj_block(b)
                    for qb in stream_qbs:
                        for qi in range(b * CPQ, (b + 1) * CPQ):
                            scores_exp_pv(qb, qi)

            # ---------------- stage 2: remaining query blocks ----------------
            flush_pv()
            pending_stream_epi = list(stream_qbs)
            # free pv0 so the first post query block can start accumulating
            epilogue(pending_stream_epi.pop(0))

            def drain_stream_epi():
                while pending_stream_epi:
                    epilogue(pending_stream_epi.pop(0))

            for qb in post_qbs:
                pv_tiles[qb] = psPV.tile([E + 1, TB], F32,
                                         tag=f"pv{qb % 2}", name=f"pv_{qb}")
                for qi in range(NT2 // CPQ):
                    scores_exp_pv(qb, qi)
                    if qi >= 1:
                        drain_stream_epi()
                drain_stream_epi()
                flush_pv()
                epilogue(qb)


def build_nc(t1l=T1L, t2=T2):
    nc = bacc.Bacc()
    qh = nc.declare_dram_parameter("qh", [P, NQB, DC, TB], BF16, isOutput=False)
    kh = nc.declare_dram_parameter("kh", [P, NBLK, DC, TB], BF16,
                                   isOutput=False)
    vh = nc.declare_dram_parameter("vh", [P, NBLK, DC, TB], BF16,
                                   isOutput=False)
    mrow = nc.declare_dram_parameter("mrow", [T2], BF16, isOutput=False)
    Wq = nc.declare_dram_parameter("Wq", [P, DC, E], BF16, isOutput=False)
    bq2 = nc.declare_dram_parameter("bq2", [P], F32, isOutput=False)
    Wk = nc.declare_dram_parameter("Wk", [P, DC, E], BF16, isOutput=False)
    bk2 = nc.declare_dram_parameter("bk2", [P], F32, isOutput=False)
    Wv = nc.declare_dram_parameter("Wv", [P, DC, E], BF16, isOutput=False)
    bv = nc.declare_dram_parameter("bv", [E], F32, isOutput=False)
    out = nc.declare_dram_parameter("out", [P, T1L // P, E], F32,
                                    isOutput=True)
    with tile.TileContext(nc) as tc:
        build_body(tc, nc, qh[:], kh[:], vh[:], mrow[:], Wq[:], bq2[:],
                   Wk[:], bk2[:], Wv[:], bv[:], out[:])
    nc.compile()
    return nc


_NC_CACHE = {}


def _get_nc():
    if "nc" not in _NC_CACHE:
        _NC_CACHE["nc"] = build_nc()
    return _NC_CACHE["nc"]


def make_in_maps(q, k, v, mask, Wq, bq, Wk, bk, Wv, bv):
    bf = ml_dtypes.bfloat16
    f32 = np.float32

    def xh(x, nb, perm):  # [t, d] fp32 -> [128, nb, 4, 512] bf16 p-major
        x = np.asarray(x, f32)
        if perm:
            x = x.reshape(nb, TB, D)[:, PERM, :].reshape(nb * TB, D)
        xt = x.astype(bf).T                              # [512, t]
        xt = xt.reshape(DC, P, nb, TB).transpose(1, 2, 0, 3)
        return np.ascontiguousarray(xt)

    def wh(W):  # [512, 64] fp32 -> [128, 4, 64] bf16 p-major
        Wr = np.asarray(W, f32).astype(bf).reshape(DC, P, E).transpose(1, 0, 2)
        return np.ascontiguousarray(Wr)

    shared = {
        "Wq": wh(Wq), "Wk": wh(Wk), "Wv": wh(Wv),
        "bq2": np.concatenate([bq, bq]).astype(f32),
        "bk2": np.concatenate([bk, bk]).astype(f32),
        "bv": np.ascontiguousarray(bv, f32),
    }
    per_b = []
    for b in range(B):
        mr = np.asarray(mask[b, 0], f32).reshape(NBLK, TB)[:, PERM]
        per_b.append({
            "kh": xh(k[b], NBLK, False),
            "vh": xh(v[b], NBLK, True),
            "mrow": np.ascontiguousarray(mr.reshape(T2).astype(bf)),
        })
    in_maps = []
    for c in range(8):
        b, h = divmod(c, 2)
        in_maps.append({
            "qh": xh(q[b, h * T1L:(h + 1) * T1L], NQB, True),
            **per_b[b],
            **shared,
        })
    return in_maps


def assemble_out(results):
    out = np.empty((B, T1, E), np.float32)
    for c in range(8):
        b, h = divmod(c, 2)
        # device out is [128, 16, 64] p-major -> [2048, 64]; the q block
        # permutation cancels exactly through the XBAR output transpose
        o = results[c]["out"].transpose(1, 0, 2).reshape(T1L, E)
        out[b, h * T1L:(h + 1) * T1L] = o
    return out


def run(inputs, trace=False):
    from concourse.bass_utils import run_bass_kernel_spmd
    _install_ntff_hook()
    nc = _get_nc()
    in_maps = make_in_maps(**inputs)
    res = run_bass_kernel_spmd(nc, in_maps, list(range(8)), trace=trace)
    return assemble_out(res.results), res


def kernel(q, k, v, mask, Wq, bq, Wk, bk, Wv, bv):
    out, _ = run(dict(q=q, k=k, v=v, mask=mask, Wq=Wq, bq=bq, Wk=Wk, bk=bk,
                      Wv=Wv, bv=bv))
    return out
